# revision 46
# baseline (speedup 1.0000x reference)
"""BandSplit layer Trainium2 kernel (fp16 I/O + PE array tiling).

Computes, for input [16, 1000, 257]:
  - 28 frequency bands: 8 bands x 4 bins (bins 0..31), 12 x 8 (32..127),
    8 x 16 (128..255)  (bin 256 unused)
  - per-band layernorm over the band's bins (eps=1e-3), with per-band
    gamma/beta, then a per-band dense projection [c] -> [128] plus bias.
  - output [16, 1000, 28, 128]

Strategy: data-parallel over batch across 8 NeuronCores (2000 tokens per
core).  The kernel is HBM-write-bound, so all device I/O is fp16 (host
casts in, upcasts out; harness tolerance 2e-2 vs fp16's ~4e-3).

The 28 bands form 9 block-diagonal matmul chunks (4 bands x 128 outputs
for the 512-col chunks, 2 bands for the 256-col ones).  The host packs
the input bins into a padded 288-col layout where each chunk's bins sit
in their own 32-col block, so after the PE transpose every chunk's
contraction rows land on a 32-aligned partition range.  Each chunk then
runs as a 32x128 row-tiled matmul (tile_position = (32-aligned row, 0)),
letting up to 4 chunks stream through the PE array concurrently instead
of paying the full 128-row dense cost of the block-diagonal matrix.

Per 128-token tile: LN stats via fp32 free-dim reduces (DVE) ->
normalize in place (Pool; it cannot access PSUM so the drains keep
DVE/Act) -> 3 PE transposes (windows 0:128 / 128:256 / 160:288) ->
9 row-tiled fp16 matmuls into 4 PSUM tiles -> PSUM->fp16 drains split
across Scalar/Vector -> 2 half-row DMAs out (4KB descriptors) on sync.
"""

import sys

import numpy as np

for _p in ("/opt/trn_rl_repo", "/root/.axon_site/_ro/trn_rl_repo"):
    if _p not in sys.path:
        sys.path.append(_p)

EPS = 1e-3
D = 128
GROUPS = [(8, 4, 0), (12, 8, 32), (8, 16, 128)]  # (n_bands, bins_per_band, start)
B, T, F = 16, 1000, 257
N_CORES = 8
TOK = B * T // N_CORES  # tokens per core = 2000
NB = 28
OUT_COLS = NB * D  # 3584
P = 128
XF = 288  # padded input cols: 9 blocks x 32

# chunk table: (row_base, K, ktile, out_col0, n_cols, [band ids])
# xpad block b = chunk b at cols 32b:32b+K; ktile windows: 0:128, 128:256,
# 160:288.  row_base = partition offset inside the ktile = PE tile row pos.
CHUNKS = [
    # group-1 blocks use an interleaved layout: band j at cols 8j:8j+4,
    # pad 8j+4:8j+8 (zero weight rows), so g1+g2 share one uniform
    # [blocks, 4, 8] normalize access pattern.
    (0, 28, 0, 0, 512, [0, 1, 2, 3]),
    (32, 28, 0, 512, 512, [4, 5, 6, 7]),
    (64, 32, 0, 1024, 512, [8, 9, 10, 11]),
    (96, 32, 0, 1536, 512, [12, 13, 14, 15]),
    (0, 32, 1, 2048, 512, [16, 17, 18, 19]),
    (32, 32, 1, 2560, 256, [20, 21]),
    (32, 32, 2, 2816, 256, [22, 23]),
    (96, 32, 1, 3072, 256, [24, 25]),
    (96, 32, 2, 3328, 256, [26, 27]),
]
# per-band (c, start_bin)
_BANDS = []
for _n, _c, _s in GROUPS:
    for _k in range(_n):
        _BANDS.append((_c, _s + _k * _c))

_STATE = {}


def _build(has_bias):
    from contextlib import ExitStack

    import concourse.bass as bass
    import concourse.tile as tile
    from concourse import bacc, mybir

    f16 = mybir.dt.float16
    f32 = mybir.dt.float32
    nc = bacc.Bacc(
        "TRN2", target_bir_lowering=False, debug=False, num_devices=N_CORES
    )
    x_d = nc.dram_tensor("x", [TOK, XF], f16, kind="ExternalInput").ap()
    w_d = nc.dram_tensor("wpack", [P, OUT_COLS], f16, kind="ExternalInput").ap()
    id_d = nc.dram_tensor("ident", [P, P], f16, kind="ExternalInput").ap()
    ci_d = nc.dram_tensor("cinv2", [1, 2 * NB], f32, kind="ExternalInput").ap()
    if has_bias:
        b_d = nc.dram_tensor("bias", [1, OUT_COLS], f32, kind="ExternalInput").ap()
    out_d = nc.dram_tensor("out", [TOK, OUT_COLS], f16, kind="ExternalOutput").ap()

    n_tiles = (TOK + P - 1) // P  # 16

    with tile.TileContext(nc) as tc, ExitStack() as ctx:
        const = ctx.enter_context(tc.tile_pool(name="const", bufs=1))
        sqp = ctx.enter_context(tc.tile_pool(name="sqp", bufs=2))
        ln = ctx.enter_context(tc.tile_pool(name="ln", bufs=2))
        xnt = ctx.enter_context(tc.tile_pool(name="xnt", bufs=3))
        outp = ctx.enter_context(tc.tile_pool(name="outp", bufs=4))
        ps_tr = ctx.enter_context(tc.tile_pool(name="ps_tr", bufs=1, space="PSUM"))
        ps_mm = ctx.enter_context(tc.tile_pool(name="ps_mm", bufs=3, space="PSUM"))
        ps_md = ctx.enter_context(tc.tile_pool(name="ps_md", bufs=1, space="PSUM"))

        # --- constants + full input prefetch ---
        xall = const.tile([P, n_tiles, XF], f16)
        # tile 0 alone first (fast trigger, ~1us to data) so the LN chain
        # starts immediately; then tiles 1-3
        nc.sync.dma_start(out=xall[:, 0, :], in_=x_d[0:128, :])
        nc.sync.dma_start(
            out=xall[:, 1:4, :],
            in_=x_d[128:512, :].rearrange("(t p) f -> p t f", p=P),
        )
        ident = const.tile([P, P], f16)
        nc.sync.dma_start(out=ident[:], in_=id_d)
        w_sbr = const.tile([P, OUT_COLS], f16)
        nc.sync.dma_start(out=w_sbr[:, 0:1024], in_=w_d[:, 0:1024])
        cinv2 = const.tile([P, 2 * NB], f32)
        nc.sync.dma_start(
            out=cinv2[:],
            in_=bass.AP(tensor=ci_d.tensor, offset=ci_d.offset, ap=[[0, P], ci_d.ap[1]]),
        )
        eps_t = const.tile([P, 1], f32)
        nc.vector.memset(eps_t[:], EPS)
        for g in range(1, 3):
            nc.sync.dma_start(
                out=xall[:, 4 * g : 4 * g + 4, :],
                in_=x_d[512 * g : 512 * (g + 1), :].rearrange(
                    "(t p) f -> p t f", p=P
                ),
            )
        nc.sync.dma_start(out=w_sbr[:, 1024:2304], in_=w_d[:, 1024:2304])
        nc.sync.dma_start(
            out=xall[:, 12:15, :],
            in_=x_d[1536:1920, :].rearrange("(t p) f -> p t f", p=P),
        )
        nc.sync.dma_start(out=xall[0:80, 15, :], in_=x_d[1920:2000, :])
        nc.sync.dma_start(out=w_sbr[:, 2304:3584], in_=w_d[:, 2304:3584])
        if has_bias:
            bias_sb = const.tile([P, OUT_COLS], f32)
            nc.sync.dma_start(
                out=bias_sb[:],
                in_=bass.AP(
                    tensor=b_d.tensor, offset=b_d.offset, ap=[[0, P], b_d.ap[1]]
                ),
            )

        # LN reduce views: (col0, ncols, blocks, g, cs, used, band0, nbands)
        # g1 bands sit at stride-8 sub-blocks with 4 valid bins each.
        RED_REGIONS = [
            (0, 64, 2, 4, 8, 4, 0, 8),
            (64, 96, 3, 4, 8, 8, 8, 12),
            (160, 128, 4, 2, 16, 16, 20, 8),
        ]
        # normalize views: g1+g2 share one uniform [blocks, 4, 8] pattern
        # (pad cols get junk, which lands on zero weight rows).
        NORM_REGIONS = [
            (0, 160, 5, 4, 8, 0, 20),
            (160, 128, 4, 2, 16, 20, 8),
        ]

        for it in range(n_tiles):
            t0 = it * P
            tn = min(P, TOK - t0)
            xt = xall[:, it, :]  # [P, XF] view

            # --- LN stats (elementwise on Pool: it cannot access PSUM, so
            # keep DVE/Act free for the PSUM drains) ---
            sq = sqp.tile([P, XF], f32, tag="sq")
            nc.gpsimd.tensor_mul(sq[:tn, :], xt[:tn, :], xt[:tn, :])

            # fp32 squares/sums: fp16 makes E[x^2]-mean^2 go negative for
            # near-zero-variance bands (sqrt -> NaN, rstd badly off).
            ss = ln.tile([P, 2, NB], f32, tag="ss")
            for half, src in ((0, xt), (1, sq)):
                for c0, ncol, nb, g, cs, used, b0, nbd in RED_REGIONS:
                    xg = (
                        src[:tn, c0 : c0 + ncol]
                        .rearrange("p (b r) -> p b r", b=nb)
                        .rearrange("p b (g c) -> p b g c", g=g)[:, :, :, 0:used]
                    )
                    og = ss[:tn, half, b0 : b0 + nbd].rearrange(
                        "p (b g) -> p b g", b=nb
                    )
                    nc.vector.reduce_sum(out=og, in_=xg, axis=mybir.AxisListType.X)

            me = ln.tile([P, 2, NB], f32, tag="me")  # mean | E[x^2]
            nc.gpsimd.tensor_mul(
                me[:tn].rearrange("p a b -> p (a b)"),
                ss[:tn].rearrange("p a b -> p (a b)"),
                cinv2[:tn],
            )
            mean = me[:, 0]
            var = ln.tile([P, NB], f32, tag="var")
            nc.gpsimd.tensor_mul(var[:tn, :], mean[:tn, :], mean[:tn, :])
            nc.gpsimd.tensor_sub(var[:tn, :], me[:tn, 1, :], var[:tn, :])
            rstd = ln.tile([P, NB], f32, tag="rstd")
            nc.scalar.activation(
                out=rstd[:tn, :],
                in_=var[:tn, :],
                func=mybir.ActivationFunctionType.Sqrt,
                bias=eps_t[:tn, :],
                scale=1.0,
            )
            nc.vector.reciprocal(out=rstd[:tn, :], in_=rstd[:tn, :])

            # --- normalize in place (Pool) ---
            for c0, ncol, nb, g, cs, b0, nbd in NORM_REGIONS:
                xg = (
                    xt[:tn, c0 : c0 + ncol]
                    .rearrange("p (b r) -> p b r", b=nb)
                    .rearrange("p b (g c) -> p b g c", g=g)
                )
                mg = (
                    mean[:tn, b0 : b0 + nbd]
                    .rearrange("p (b g) -> p b g", b=nb)
                    .to_broadcast((tn, nb, g, cs))
                )
                rg = (
                    rstd[:tn, b0 : b0 + nbd]
                    .rearrange("p (b g) -> p b g", b=nb)
                    .to_broadcast((tn, nb, g, cs))
                )
                nc.gpsimd.tensor_sub(xg, xg, mg)
                nc.gpsimd.tensor_mul(xg, xg, rg)

            # --- 3 transposes to [bins, tok] ---
            pt = ps_tr.tile([P, 3 * P], f16, tag="pt")
            for k, w0 in enumerate((0, 128, 160)):
                nc.tensor.transpose(
                    pt[:, k * P : k * P + tn],
                    xt[:tn, w0 : w0 + P],
                    ident[:tn, :tn],
                )
            st = xnt.tile([P, 3 * P], f16, tag="st")
            if tn == P:
                nc.vector.tensor_copy(st[:, :], pt[:, :])
            else:
                for k in range(3):
                    nc.vector.tensor_copy(
                        st[:, k * P : k * P + tn], pt[:, k * P : k * P + tn]
                    )

            # --- 9 row-tiled matmuls into 4 PSUM tiles ---
            pmA = ps_mm.tile([P, 1024], f32, tag="pm")
            pmB = ps_mm.tile([P, 1024], f32, tag="pm")
            pmC = ps_mm.tile([P, 1024], f32, tag="pm")
            pmD = ps_md.tile([P, 512], f32, tag="pmd")
            # chunk -> (psum tile, col offset inside it)
            DEST = [
                (pmA, 0), (pmA, 512), (pmB, 0), (pmB, 512),
                (pmC, 0), (pmC, 512), (pmC, 768), (pmD, 0), (pmD, 256),
            ]
            for ci, (rb, K, kt, oc0, ncols, _bs) in enumerate(CHUNKS):
                pm, po = DEST[ci]
                nc.tensor.matmul(
                    pm[:tn, po : po + ncols],
                    st[rb : rb + K, kt * P : kt * P + tn],
                    w_sbr[rb : rb + K, oc0 : oc0 + ncols],
                    start=True,
                    stop=True,
                    tile_position=(rb, 0),
                )

            # --- drains (PSUM -> fp16 SBUF) + half-row DMAs out ---
            ot = outp.tile([P, OUT_COLS], f16, tag="ot")

            def drain(eng, pm, po, c0, n):
                osl = ot[:tn, c0 : c0 + n]
                pms = pm[:tn, po : po + n]
                if has_bias:
                    # Act engine can't tensor_add a full bias row; reroute.
                    e = nc.vector if eng is nc.scalar else eng
                    e.tensor_add(osl, pms, bias_sb[:tn, c0 : c0 + n])
                elif eng is nc.scalar:
                    nc.scalar.copy(osl, pms)
                else:
                    eng.tensor_copy(osl, pms)

            drain(nc.scalar, pmA, 0, 0, 1024)
            drain(nc.vector, pmB, 0, 1024, 1024)
            nc.sync.dma_start(out=out_d[t0 : t0 + tn, 0:2048], in_=ot[:tn, 0:2048])
            drain(nc.scalar, pmC, 0, 2048, 1024)
            drain(nc.vector if it % 2 == 0 else nc.scalar, pmD, 0, 3072, 512)
            nc.sync.dma_start(
                out=out_d[t0 : t0 + tn, 2048:3584], in_=ot[:tn, 2048:3584]
            )

    nc.compile()
    return nc


def _get_nc(has_bias):
    key = ("nc", has_bias)
    if key not in _STATE:
        _STATE[key] = _build(has_bias)
    return _STATE[key]


def _pack_weights(inputs):
    """Fold gamma into W, beta/b into bias; pack per-chunk rows (fp16)."""
    wpack = np.zeros((P, OUT_COLS), dtype=np.float32)
    bias = np.zeros((OUT_COLS,), dtype=np.float32)
    gammas, betas, Ws, bs = [], [], [], []
    for gi in range(1, 4):
        gammas.append(np.asarray(inputs[f"gamma{gi}"], dtype=np.float32))
        betas.append(np.asarray(inputs[f"beta{gi}"], dtype=np.float32))
        Ws.append(np.asarray(inputs[f"W{gi}"], dtype=np.float32))
        bs.append(np.asarray(inputs[f"b{gi}"], dtype=np.float32))
    gl = [g for gm in gammas for g in gm]
    bl = [b for bm in betas for b in bm]
    Wl = [w for wm in Ws for w in wm]
    bbl = [b for bm in bs for b in bm]
    for rb, K, kt, oc0, ncols, bands in CHUNKS:
        for j, bi in enumerate(bands):
            c, _s = _BANDS[bi]
            r0 = rb + j * (8 if c == 4 else c)  # g1: interleaved stride-8
            c0 = oc0 + j * D
            wpack[r0 : r0 + c, c0 : c0 + D] = gl[bi][:, None] * Wl[bi]
            bias[c0 : c0 + D] = bl[bi] @ Wl[bi] + bbl[bi]
    return wpack.astype(np.float16), bias


def _pack_x(x):
    """[B*T, 257] fp32 -> padded fp16 [B*T, 288] per CHUNKS layout."""
    xp = np.zeros((x.shape[0], XF), dtype=np.float16)
    for bidx, (rb, K, kt, oc0, ncols, bands) in enumerate(CHUNKS):
        col = 32 * bidx
        for j, bi in enumerate(bands):
            c, s = _BANDS[bi]
            o = col + j * (8 if c == 4 else c)  # g1: interleaved stride-8
            xp[:, o : o + c] = x[:, s : s + c]
    return xp


def _cinv2():
    ci = np.zeros((1, 2 * NB), dtype=np.float32)
    for half in range(2):
        b0 = 0
        for n, c, _s in GROUPS:
            ci[0, half * NB + b0 : half * NB + b0 + n] = 1.0 / c
            b0 += n
    return ci


def _prepare(inputs):
    x = np.asarray(inputs["inputs"], dtype=np.float32)
    assert x.shape == (B, T, F), x.shape
    wpack, bias = _pack_weights(inputs)
    has_bias = bool(np.any(bias != 0.0))

    nc = _get_nc(has_bias)

    xpad = _pack_x(np.ascontiguousarray(x.reshape(B * T, F)))
    ident = np.eye(P, dtype=np.float16)
    cinv2 = _cinv2()
    in_maps = []
    for c in range(N_CORES):
        m = {
            "x": xpad[c * TOK : (c + 1) * TOK],
            "wpack": wpack,
            "ident": ident,
            "cinv2": cinv2,
        }
        if has_bias:
            m["bias"] = bias.reshape(1, OUT_COLS).astype(np.float32)
        in_maps.append(m)
    return nc, in_maps


def kernel(**inputs):
    from concourse.bass_utils import run_bass_kernel_spmd

    nc, in_maps = _prepare(inputs)
    res = run_bass_kernel_spmd(nc, in_maps, list(range(N_CORES))).results
    out = np.concatenate([r["out"] for r in res], axis=0)
    return out.astype(np.float32).reshape(B, T, NB, D)


# revision 47
# speedup vs baseline: 1.1646x; 1.1646x over previous
"""BandSplit layer Trainium2 kernel (fp16 I/O + PE array tiling).

Computes, for input [16, 1000, 257]:
  - 28 frequency bands: 8 bands x 4 bins (bins 0..31), 12 x 8 (32..127),
    8 x 16 (128..255)  (bin 256 unused)
  - per-band layernorm over the band's bins (eps=1e-3), with per-band
    gamma/beta, then a per-band dense projection [c] -> [128] plus bias.
  - output [16, 1000, 28, 128]

Strategy: data-parallel over batch across 8 NeuronCores (2000 tokens per
core).  The kernel is HBM-write-bound, so all device I/O is fp16 (host
casts in, upcasts out; harness tolerance 2e-2 vs fp16's ~4e-3).

The 28 bands form 9 block-diagonal matmul chunks (4 bands x 128 outputs
for the 512-col chunks, 2 bands for the 256-col ones).  The host packs
the input bins into a padded 288-col layout where each chunk's bins sit
in their own 32-col block, so after the PE transpose every chunk's
contraction rows land on a 32-aligned partition range.  Each chunk then
runs as a 32x128 row-tiled matmul (tile_position = (32-aligned row, 0)),
letting up to 4 chunks stream through the PE array concurrently instead
of paying the full 128-row dense cost of the block-diagonal matrix.

Per 128-token tile: LN stats via fp32 free-dim reduces (DVE) ->
normalize in place (Pool; it cannot access PSUM so the drains keep
DVE/Act) -> 3 PE transposes (windows 0:128 / 128:256 / 160:288) ->
9 row-tiled fp16 matmuls into 4 PSUM tiles -> PSUM->fp16 drains split
across Scalar/Vector -> 2 half-row DMAs out (4KB descriptors) on sync.
"""

import sys

import numpy as np

for _p in ("/opt/trn_rl_repo", "/root/.axon_site/_ro/trn_rl_repo"):
    if _p not in sys.path:
        sys.path.append(_p)

EPS = 1e-3
D = 128
GROUPS = [(8, 4, 0), (12, 8, 32), (8, 16, 128)]  # (n_bands, bins_per_band, start)
B, T, F = 16, 1000, 257
N_CORES = 8
TOK = B * T // N_CORES  # tokens per core = 2000
NB = 28
OUT_COLS = NB * D  # 3584
P = 128
XF = 288  # padded input cols: 9 blocks x 32

# chunk table: (row_base, K, ktile, out_col0, n_cols, [band ids])
# xpad block b = chunk b at cols 32b:32b+K; ktile windows: 0:128, 128:256,
# 160:288.  row_base = partition offset inside the ktile = PE tile row pos.
CHUNKS = [
    (0, 16, 0, 0, 512, [0, 1, 2, 3]),
    (32, 16, 0, 512, 512, [4, 5, 6, 7]),
    (64, 32, 0, 1024, 512, [8, 9, 10, 11]),
    (96, 32, 0, 1536, 512, [12, 13, 14, 15]),
    (0, 32, 1, 2048, 512, [16, 17, 18, 19]),
    (32, 32, 1, 2560, 256, [20, 21]),
    (32, 32, 2, 2816, 256, [22, 23]),
    (96, 32, 1, 3072, 256, [24, 25]),
    (96, 32, 2, 3328, 256, [26, 27]),
]
# per-band (c, start_bin)
_BANDS = []
for _n, _c, _s in GROUPS:
    for _k in range(_n):
        _BANDS.append((_c, _s + _k * _c))

_STATE = {}


def _build(has_bias):
    from contextlib import ExitStack

    import concourse.bass as bass
    import concourse.tile as tile
    from concourse import bacc, mybir

    f16 = mybir.dt.float16
    f32 = mybir.dt.float32
    nc = bacc.Bacc(
        "TRN2", target_bir_lowering=False, debug=False, num_devices=N_CORES
    )
    x_d = nc.dram_tensor("x", [TOK, XF], f16, kind="ExternalInput").ap()
    w_d = nc.dram_tensor("wpack", [P, OUT_COLS], f16, kind="ExternalInput").ap()
    id_d = nc.dram_tensor("ident", [P, P], f16, kind="ExternalInput").ap()
    ci_d = nc.dram_tensor("cinv2", [1, 2 * NB], f32, kind="ExternalInput").ap()
    if has_bias:
        b_d = nc.dram_tensor("bias", [1, OUT_COLS], f32, kind="ExternalInput").ap()
    out_d = nc.dram_tensor("out", [TOK, OUT_COLS], f16, kind="ExternalOutput").ap()

    n_tiles = (TOK + P - 1) // P  # 16

    with tile.TileContext(nc) as tc, ExitStack() as ctx:
        const = ctx.enter_context(tc.tile_pool(name="const", bufs=1))
        sqp = ctx.enter_context(tc.tile_pool(name="sqp", bufs=2))
        ln = ctx.enter_context(tc.tile_pool(name="ln", bufs=2))
        xnt = ctx.enter_context(tc.tile_pool(name="xnt", bufs=3))
        outp = ctx.enter_context(tc.tile_pool(name="outp", bufs=4))
        ps_tr = ctx.enter_context(tc.tile_pool(name="ps_tr", bufs=1, space="PSUM"))
        ps_mm = ctx.enter_context(tc.tile_pool(name="ps_mm", bufs=3, space="PSUM"))
        ps_md = ctx.enter_context(tc.tile_pool(name="ps_md", bufs=1, space="PSUM"))

        # --- constants + full input prefetch ---
        xall = const.tile([P, n_tiles, XF], f16)
        # tile 0 alone first (fast trigger, ~1us to data) so the LN chain
        # starts immediately; then tiles 1-3
        nc.sync.dma_start(out=xall[:, 0, :], in_=x_d[0:128, :])
        nc.sync.dma_start(
            out=xall[:, 1:4, :],
            in_=x_d[128:512, :].rearrange("(t p) f -> p t f", p=P),
        )
        ident = const.tile([P, P], f16)
        nc.sync.dma_start(out=ident[:], in_=id_d)
        w_sbr = const.tile([P, OUT_COLS], f16)
        nc.sync.dma_start(out=w_sbr[:, 0:1024], in_=w_d[:, 0:1024])
        cinv2 = const.tile([P, 2 * NB], f32)
        nc.sync.dma_start(
            out=cinv2[:],
            in_=bass.AP(tensor=ci_d.tensor, offset=ci_d.offset, ap=[[0, P], ci_d.ap[1]]),
        )
        eps_t = const.tile([P, 1], f32)
        nc.vector.memset(eps_t[:], EPS)
        for g in range(1, 3):
            nc.sync.dma_start(
                out=xall[:, 4 * g : 4 * g + 4, :],
                in_=x_d[512 * g : 512 * (g + 1), :].rearrange(
                    "(t p) f -> p t f", p=P
                ),
            )
        nc.sync.dma_start(out=w_sbr[:, 1024:2304], in_=w_d[:, 1024:2304])
        nc.sync.dma_start(
            out=xall[:, 12:15, :],
            in_=x_d[1536:1920, :].rearrange("(t p) f -> p t f", p=P),
        )
        nc.sync.dma_start(out=xall[0:80, 15, :], in_=x_d[1920:2000, :])
        nc.sync.dma_start(out=w_sbr[:, 2304:3584], in_=w_d[:, 2304:3584])
        if has_bias:
            bias_sb = const.tile([P, OUT_COLS], f32)
            nc.sync.dma_start(
                out=bias_sb[:],
                in_=bass.AP(
                    tensor=b_d.tensor, offset=b_d.offset, ap=[[0, P], b_d.ap[1]]
                ),
            )

        # LN region views: (col0, ncols, blocks, used, g, c, band0, nbands)
        REGIONS = [
            (0, 64, 2, 16, 4, 4, 0, 8),
            (64, 96, 3, 32, 4, 8, 8, 12),
            (160, 128, 4, 32, 2, 16, 20, 8),
        ]

        for it in range(n_tiles):
            t0 = it * P
            tn = min(P, TOK - t0)
            xt = xall[:, it, :]  # [P, XF] view

            # --- LN stats (elementwise on Pool: it cannot access PSUM, so
            # keep DVE/Act free for the PSUM drains) ---
            sq = sqp.tile([P, XF], f32, tag="sq")
            nc.gpsimd.tensor_mul(sq[:tn, :], xt[:tn, :], xt[:tn, :])

            # fp32 squares/sums: fp16 makes E[x^2]-mean^2 go negative for
            # near-zero-variance bands (sqrt -> NaN, rstd badly off).
            ss = ln.tile([P, 2, NB], f32, tag="ss")
            for half, src in ((0, xt), (1, sq)):
                for c0, ncol, nb, used, g, c, b0, nbd in REGIONS:
                    xg = (
                        src[:tn, c0 : c0 + ncol]
                        .rearrange("p (b r) -> p b r", b=nb)[:, :, 0:used]
                        .rearrange("p b (g c) -> p b g c", g=g)
                    )
                    og = ss[:tn, half, b0 : b0 + nbd].rearrange(
                        "p (b g) -> p b g", b=nb
                    )
                    nc.vector.reduce_sum(out=og, in_=xg, axis=mybir.AxisListType.X)

            me = ln.tile([P, 2, NB], f32, tag="me")  # mean | E[x^2]
            nc.gpsimd.tensor_mul(
                me[:tn].rearrange("p a b -> p (a b)"),
                ss[:tn].rearrange("p a b -> p (a b)"),
                cinv2[:tn],
            )
            mean = me[:, 0]
            var = ln.tile([P, NB], f32, tag="var")
            nc.gpsimd.tensor_mul(var[:tn, :], mean[:tn, :], mean[:tn, :])
            nc.gpsimd.tensor_sub(var[:tn, :], me[:tn, 1, :], var[:tn, :])
            rstd = ln.tile([P, NB], f32, tag="rstd")
            nc.scalar.activation(
                out=rstd[:tn, :],
                in_=var[:tn, :],
                func=mybir.ActivationFunctionType.Sqrt,
                bias=eps_t[:tn, :],
                scale=1.0,
            )
            nc.vector.reciprocal(out=rstd[:tn, :], in_=rstd[:tn, :])

            # --- normalize in place (Pool) ---
            for c0, ncol, nb, used, g, c, b0, nbd in REGIONS:
                xg = (
                    xt[:tn, c0 : c0 + ncol]
                    .rearrange("p (b r) -> p b r", b=nb)[:, :, 0:used]
                    .rearrange("p b (g c) -> p b g c", g=g)
                )
                mg = (
                    mean[:tn, b0 : b0 + nbd]
                    .rearrange("p (b g) -> p b g", b=nb)
                    .to_broadcast((tn, nb, g, c))
                )
                rg = (
                    rstd[:tn, b0 : b0 + nbd]
                    .rearrange("p (b g) -> p b g", b=nb)
                    .to_broadcast((tn, nb, g, c))
                )
                nc.gpsimd.tensor_sub(xg, xg, mg)
                nc.gpsimd.tensor_mul(xg, xg, rg)

            # --- 3 transposes to [bins, tok] ---
            pt = ps_tr.tile([P, 3 * P], f16, tag="pt")
            for k, w0 in enumerate((0, 128, 160)):
                nc.tensor.transpose(
                    pt[:, k * P : k * P + tn],
                    xt[:tn, w0 : w0 + P],
                    ident[:tn, :tn],
                )
            st = xnt.tile([P, 3 * P], f16, tag="st")
            if tn == P:
                nc.vector.tensor_copy(st[:, :], pt[:, :])
            else:
                for k in range(3):
                    nc.vector.tensor_copy(
                        st[:, k * P : k * P + tn], pt[:, k * P : k * P + tn]
                    )

            # --- 9 row-tiled matmuls into 4 PSUM tiles ---
            pmA = ps_mm.tile([P, 1024], f32, tag="pm")
            pmB = ps_mm.tile([P, 1024], f32, tag="pm")
            pmC = ps_mm.tile([P, 1024], f32, tag="pm")
            pmD = ps_md.tile([P, 512], f32, tag="pmd")
            # chunk -> (psum tile, col offset inside it)
            DEST = [
                (pmA, 0), (pmA, 512), (pmB, 0), (pmB, 512),
                (pmC, 0), (pmC, 512), (pmC, 768), (pmD, 0), (pmD, 256),
            ]
            for ci, (rb, K, kt, oc0, ncols, _bs) in enumerate(CHUNKS):
                pm, po = DEST[ci]
                nc.tensor.matmul(
                    pm[:tn, po : po + ncols],
                    st[rb : rb + K, kt * P : kt * P + tn],
                    w_sbr[rb : rb + K, oc0 : oc0 + ncols],
                    start=True,
                    stop=True,
                    tile_position=(rb, 0),
                )

            # --- drains (PSUM -> fp16 SBUF) + half-row DMAs out ---
            ot = outp.tile([P, OUT_COLS], f16, tag="ot")

            def drain(eng, pm, po, c0, n):
                osl = ot[:tn, c0 : c0 + n]
                pms = pm[:tn, po : po + n]
                if has_bias:
                    # Act engine can't tensor_add a full bias row; reroute.
                    e = nc.vector if eng is nc.scalar else eng
                    e.tensor_add(osl, pms, bias_sb[:tn, c0 : c0 + n])
                elif eng is nc.scalar:
                    nc.scalar.copy(osl, pms)
                else:
                    eng.tensor_copy(osl, pms)

            drain(nc.scalar, pmA, 0, 0, 1024)
            drain(nc.vector, pmB, 0, 1024, 1024)
            nc.sync.dma_start(out=out_d[t0 : t0 + tn, 0:2048], in_=ot[:tn, 0:2048])
            drain(nc.scalar, pmC, 0, 2048, 1024)
            drain(nc.vector if it % 2 == 0 else nc.scalar, pmD, 0, 3072, 512)
            nc.sync.dma_start(
                out=out_d[t0 : t0 + tn, 2048:3584], in_=ot[:tn, 2048:3584]
            )

    nc.compile()
    return nc


def _get_nc(has_bias):
    key = ("nc", has_bias)
    if key not in _STATE:
        _STATE[key] = _build(has_bias)
    return _STATE[key]


def _pack_weights(inputs):
    """Fold gamma into W, beta/b into bias; pack per-chunk rows (fp16)."""
    wpack = np.zeros((P, OUT_COLS), dtype=np.float32)
    bias = np.zeros((OUT_COLS,), dtype=np.float32)
    gammas, betas, Ws, bs = [], [], [], []
    for gi in range(1, 4):
        gammas.append(np.asarray(inputs[f"gamma{gi}"], dtype=np.float32))
        betas.append(np.asarray(inputs[f"beta{gi}"], dtype=np.float32))
        Ws.append(np.asarray(inputs[f"W{gi}"], dtype=np.float32))
        bs.append(np.asarray(inputs[f"b{gi}"], dtype=np.float32))
    gl = [g for gm in gammas for g in gm]
    bl = [b for bm in betas for b in bm]
    Wl = [w for wm in Ws for w in wm]
    bbl = [b for bm in bs for b in bm]
    for rb, K, kt, oc0, ncols, bands in CHUNKS:
        for j, bi in enumerate(bands):
            c, _s = _BANDS[bi]
            r0 = rb + j * c
            c0 = oc0 + j * D
            wpack[r0 : r0 + c, c0 : c0 + D] = gl[bi][:, None] * Wl[bi]
            bias[c0 : c0 + D] = bl[bi] @ Wl[bi] + bbl[bi]
    return wpack.astype(np.float16), bias


def _pack_x(x):
    """[B*T, 257] fp32 -> padded fp16 [B*T, 288] per CHUNKS layout."""
    xp = np.zeros((x.shape[0], XF), dtype=np.float16)
    for bidx, (rb, K, kt, oc0, ncols, bands) in enumerate(CHUNKS):
        col = 32 * bidx
        for j, bi in enumerate(bands):
            c, s = _BANDS[bi]
            xp[:, col + j * c : col + (j + 1) * c] = x[:, s : s + c]
    return xp


def _cinv2():
    ci = np.zeros((1, 2 * NB), dtype=np.float32)
    for half in range(2):
        b0 = 0
        for n, c, _s in GROUPS:
            ci[0, half * NB + b0 : half * NB + b0 + n] = 1.0 / c
            b0 += n
    return ci


def _prepare(inputs):
    x = np.asarray(inputs["inputs"], dtype=np.float32)
    assert x.shape == (B, T, F), x.shape
    wpack, bias = _pack_weights(inputs)
    has_bias = bool(np.any(bias != 0.0))

    nc = _get_nc(has_bias)

    xpad = _pack_x(np.ascontiguousarray(x.reshape(B * T, F)))
    ident = np.eye(P, dtype=np.float16)
    cinv2 = _cinv2()
    in_maps = []
    for c in range(N_CORES):
        m = {
            "x": xpad[c * TOK : (c + 1) * TOK],
            "wpack": wpack,
            "ident": ident,
            "cinv2": cinv2,
        }
        if has_bias:
            m["bias"] = bias.reshape(1, OUT_COLS).astype(np.float32)
        in_maps.append(m)
    return nc, in_maps


def kernel(**inputs):
    from concourse.bass_utils import run_bass_kernel_spmd

    nc, in_maps = _prepare(inputs)
    res = run_bass_kernel_spmd(nc, in_maps, list(range(N_CORES))).results
    out = np.concatenate([r["out"] for r in res], axis=0)
    return out.astype(np.float32).reshape(B, T, NB, D)
